# revision 16
# baseline (speedup 1.0000x reference)
"""GAT head (masked row-softmax attention + aggregation) on 8 TRN2 NeuronCores.

Sharding: rows of the NxN attention matrix are split across 8 cores (1024
each). w/a are replicated. h is computed cooperatively: every core computes
h-rows for its own 1024 nodes and AllGathers them in two halves (so the main
loop can consume the first half while the second is still in flight); nodes
0..NLOC*128-1 are additionally computed locally by every core to cover the
collective's latency.

Per-core layout: attention tiles are held transposed [j, i] (j = neighbour on
partitions, i = own rows on the free dim) so the aggregation
u.T[f, i] += h_aug[j, f].T @ p[j, i] contracts over j on the PE directly, with
a ones-column of h_aug producing the softmax row-sums for free.

Per 128-neighbour chunk, one of three schemes computes
p = exp(leaky_relu(s_src[i] + s_dst[j])) * adj[j, i], chosen per chunk to
balance ACT vs DVE vs DMA load:
  za: ACT Prelu(bc_src + bias sd) -> SWDGE fp8 accum-DMA adds {0,-57344}
      -> ACT Exp (f32r out)
  zd: DVE STT (cmpl*-57344 + bc_src) -> ACT Prelu(bias sd) -> ACT Exp
  yd: DVE TS min-fold + 2x DVE STT (leaky_relu + mask without ACT)
      -> ACT Exp(bias sd)
"""
import os

import numpy as np
import ml_dtypes

N_NODES = 8192
D_IN = 512
F_OUT = 64
N_CORES = 8
R = N_NODES // N_CORES          # 1024 attention rows per core
NCHUNK = N_NODES // 128         # 64 j-chunks
DCHUNK = D_IN // 128            # 4 contraction chunks
NLOC = 32                       # j-chunks computed locally (nodes 0..4095)
BIG = 57344.0                   # |fp8e5 0xFB|

# per-chunk scheme: during the collective window avoid SWDGE accum-DMAs
# (zd/yd); after it, use the DMA-masked schemes (za/ya) to offload DVE/ACT.
PATTERN = (["zd", "zd", "yd", "zd", "zd", "zd", "yd", "zd", "zd", "yd"] * 7)[:NCHUNK]

LAST_EXEC_NS = None
_CACHE = {}


def _gside(c):
    return 0 if (c % 8) < 4 else 1


def _grow(c):
    return (c // 8) * 512 + ((c % 8) - 4 * _gside(c)) * 128


def _build():
    import concourse.bacc as bacc
    import concourse.mybir as mybir
    import concourse.tile as tile
    from concourse.masks import make_identity

    F32 = mybir.dt.float32
    F32R = mybir.dt.float32r
    FP8E5 = mybir.dt.float8e5
    U8 = mybir.dt.uint8
    AF = mybir.ActivationFunctionType
    OP = mybir.AluOpType
    FA = F_OUT + 2              # h_aug width: h(64) | s_dst | ones

    nc = bacc.Bacc("TRN2", target_bir_lowering=False, debug=False,
                   num_devices=N_CORES)

    inpre = nc.dram_tensor("inpre", [D_IN, NLOC * 128], F32, kind="ExternalInput")
    myinT = nc.dram_tensor("myinT", [D_IN, R], F32, kind="ExternalInput")
    adju8 = nc.dram_tensor("adju8", [N_NODES, R], U8, kind="ExternalInput")
    maskf8 = nc.dram_tensor("maskf8", [N_NODES, R], FP8E5, kind="ExternalInput")
    w = nc.dram_tensor("w", [D_IN, F_OUT], F32, kind="ExternalInput")
    wT = nc.dram_tensor("wT", [F_OUT, D_IN], F32, kind="ExternalInput")
    a2 = nc.dram_tensor("a2", [F_OUT, 2], F32, kind="ExternalInput")
    outd = nc.dram_tensor("out", [R, F_OUT], F32, kind="ExternalOutput")

    with tile.TileContext(nc) as tc:
        with tc.tile_pool(name="const", bufs=1) as const, \
             tc.tile_pool(name="haug", bufs=1) as haug, \
             tc.tile_pool(name="inp", bufs=3) as inp, \
             tc.tile_pool(name="hts", bufs=2) as hts, \
             tc.tile_pool(name="adjp", bufs=8) as adjp, \
             tc.tile_pool(name="tq", bufs=14) as tq, \
             tc.tile_pool(name="qq", bufs=5) as qq, \
             tc.tile_pool(name="ep", bufs=2) as ep, \
             tc.tile_pool(name="dram", bufs=1, space="DRAM") as dram, \
             tc.tile_pool(name="psU", bufs=1, space="PSUM") as psU, \
             tc.tile_pool(name="psH", bufs=2, space="PSUM") as psH, \
             tc.tile_pool(name="psT", bufs=2, space="PSUM") as psT, \
             tc.tile_pool(name="psS", bufs=2, space="PSUM") as psS:

            # ---- constants -------------------------------------------------
            ident = const.tile([128, 128], F32, tag="ident")
            make_identity(nc, ident)
            ones_r = const.tile([1, 128], F32, tag="ones_r")
            nc.vector.memset(ones_r, 1.0)
            alpha = const.tile([128, 1], F32, tag="alpha")
            nc.vector.memset(alpha, 0.2)
            ones_c = const.tile([128, 1], F32, tag="ones_c")
            nc.vector.memset(ones_c, 1.0)
            wT_sb = const.tile([F_OUT, D_IN], F32, tag="wT_sb")
            nc.sync.dma_start(out=wT_sb, in_=wT[:, :])
            a2_sb = const.tile([F_OUT, 2], F32, tag="a2_sb")
            nc.sync.dma_start(out=a2_sb, in_=a2[:, :])

            # own-row input tiles (shared by s-phase and my-h phase)
            my_tiles = []
            for t2 in range(R // 512):
                mts = []
                for dc in range(DCHUNK):
                    mi = inp.tile([128, 512], F32, tag=f"minp{t2}_{dc}",
                                  name=f"minp{t2}_{dc}")
                    nc.sync.dma_start(
                        out=mi, in_=myinT[dc * 128:(dc + 1) * 128,
                                          t2 * 512:(t2 + 1) * 512])
                    mts.append(mi)
                my_tiles.append(mts)

            # ---- wa = w @ [a_src | a_dst] ----------------------------------
            w_aug = []
            for dc in range(DCHUNK):
                pwa = psS.tile([128, 2], F32, tag="small")
                nc.tensor.matmul(pwa, wT_sb[:, dc * 128:(dc + 1) * 128], a2_sb,
                                 start=True, stop=True)
                wa = const.tile([128, F_OUT + 1], F32, tag=f"waug{dc}",
                                name=f"waug{dc}")
                nc.sync.dma_start(out=wa[:, 0:F_OUT],
                                  in_=w[dc * 128:(dc + 1) * 128, :])
                nc.vector.tensor_copy(wa[:, F_OUT:F_OUT + 1], pwa[:, 1:2])
                w_aug.append(wa)

            def make_haug_tiles(hT_sb, chunk_ids, sink):
                # hT_sb: [65, 512] fp32 h.T slab; emit 4 transposed tiles
                for k2, c in enumerate(chunk_ids):
                    tr = psT.tile([128, FA], F32, tag="tr")
                    nc.tensor.transpose(
                        tr[:, 0:F_OUT + 1],
                        hT_sb[:, k2 * 128:(k2 + 1) * 128],
                        ident[0:F_OUT + 1, 0:F_OUT + 1])
                    sink(c, tr)

            # ---- s_src for own rows (from own h.T slabs) -------------------
            s_row = const.tile([1, R], F32, tag="s_row")
            # ---- own h-block -> DRAM halves, two AllGathers ---------------
            hblk = [dram.tile([512, FA], F32, name=f"hblk{g}") for g in range(2)]
            gfull = [dram.tile([N_NODES // 2, FA], F32, addr_space="Shared",
                               name=f"gfull{g}") for g in range(2)]
            for t2 in range(R // 512):
                phm = psH.tile([F_OUT + 1, 512], F32, tag="hT")
                for dc in range(DCHUNK):
                    nc.tensor.matmul(phm, w_aug[dc], my_tiles[t2][dc],
                                     start=(dc == 0), stop=(dc == DCHUNK - 1))
                hTm = hts.tile([F_OUT + 1, 512], F32, tag="hT_sb")
                nc.vector.tensor_copy(hTm, phm)
                pss = psS.tile([1, 512], F32, tag="small")
                nc.tensor.matmul(pss, a2_sb[:, 0:1], hTm[0:F_OUT, :],
                                 start=True, stop=True)
                nc.vector.tensor_copy(s_row[0:1, t2 * 512:(t2 + 1) * 512],
                                      pss[0:1, :])

                def store_sink(m, tr, t2=t2):
                    hb = hts.tile([128, FA], F32, tag="hb")
                    nc.vector.tensor_copy(hb[:, 0:F_OUT + 1],
                                          tr[:, 0:F_OUT + 1])
                    nc.vector.tensor_copy(hb[:, F_OUT + 1:FA], ones_c)
                    nc.sync.dma_start(
                        out=hblk[t2][m * 128:(m + 1) * 128, :], in_=hb)

                make_haug_tiles(hTm, list(range(4)), store_sink)
                nc.gpsimd.collective_compute(
                    "AllGather", OP.bypass,
                    replica_groups=[list(range(N_CORES))],
                    ins=[hblk[t2][:, :].opt()], outs=[gfull[t2][:, :].opt()])


            bc_src = const.tile([128, R], F32, tag="bc_src")
            for t2 in range(R // 512):
                pbc = psS.tile([128, 512], F32, tag="small")
                nc.tensor.matmul(pbc, ones_r,
                                 s_row[0:1, t2 * 512:(t2 + 1) * 512],
                                 start=True, stop=True)
                nc.vector.tensor_copy(bc_src[:, t2 * 512:(t2 + 1) * 512], pbc)

            # ---- persistent per-chunk tiles --------------------------------
            h_aug = [haug.tile([128, FA], F32R, tag=f"h{c}", name=f"h{c}")
                     for c in range(NCHUNK)]
            sd_lo = [const.tile([128, 1], F32, tag=f"sd{c}", name=f"sd{c}")
                     for c in range(NLOC)]
            sd_g = [const.tile([128, 32], F32, tag=f"sdg{g}", name=f"sdg{g}")
                    for g in range(2)]
            u_ps = [psU.tile([FA, 512], F32, tag=f"u{h2}", name=f"u{h2}")
                    for h2 in range(2)]

            def local_sink(c, tr):
                nc.vector.tensor_copy(h_aug[c][:, 0:F_OUT + 1],
                                      tr[:, 0:F_OUT + 1])
                nc.vector.tensor_copy(h_aug[c][:, F_OUT + 1:FA], ones_c)
                nc.vector.tensor_copy(sd_lo[c], tr[:, F_OUT:F_OUT + 1])

            # ---- local h for n-tile 0 (fast start for chunks 0..3) --------
            def local_htile(t):
                ph = psH.tile([F_OUT + 1, 512], F32, tag="hT")
                for dc in range(DCHUNK):
                    it = inp.tile([128, 512], F32, tag="inp")
                    nc.sync.dma_start(
                        out=it, in_=inpre[dc * 128:(dc + 1) * 128,
                                          t * 512:(t + 1) * 512])
                    nc.tensor.matmul(ph, w_aug[dc], it,
                                     start=(dc == 0), stop=(dc == DCHUNK - 1))
                hT_sb = hts.tile([F_OUT + 1, 512], F32, tag="hT_sb")
                nc.vector.tensor_copy(hT_sb, ph)
                make_haug_tiles(hT_sb, [4 * t + k2 for k2 in range(4)],
                                local_sink)

            # ---- main loop helpers ---------------------------------------
            def sd_ap(c):
                if c < NLOC:
                    return sd_lo[c][:, 0:1]
                r = (c // 8) * 4 + (c % 8) - 4 * _gside(c)
                return sd_g[_gside(c)][:, r:r + 1]

            emit_idx = [0]

            def main_chunk(c):
                first = emit_idx[0] == 0
                last = emit_idx[0] == NCHUNK - 1
                emit_idx[0] += 1
                kind = PATTERN[c]
                if kind == "za":
                    tt = tq.tile([128, R], F32, tag="t")
                    nc.scalar.activation(tt, bc_src, AF.Prelu,
                                         bias=sd_ap(c), scale=1.0,
                                         alpha=alpha[:, 0:1])
                    nc.gpsimd.dma_start(
                        out=tt, in_=maskf8[c * 128:(c + 1) * 128, :],
                        accum_op=OP.add)
                    qt = qq.tile([128, R], F32R, tag="q")
                    nc.scalar.activation(qt, tt, AF.Exp, bias=0.0, scale=1.0)
                elif kind == "zd":
                    at = adjp.tile([128, R], U8, tag="adj")
                    nc.sync.dma_start(out=at,
                                      in_=adju8[c * 128:(c + 1) * 128, :])
                    t0 = tq.tile([128, R], F32, tag="t")
                    nc.vector.scalar_tensor_tensor(
                        t0, at, -BIG, bc_src, op0=OP.mult, op1=OP.add)
                    t1 = tq.tile([128, R], F32, tag="t")
                    nc.scalar.activation(t1, t0, AF.Prelu,
                                         bias=sd_ap(c), scale=1.0,
                                         alpha=alpha[:, 0:1])
                    qt = qq.tile([128, R], F32R, tag="q")
                    nc.scalar.activation(qt, t1, AF.Exp, bias=0.0, scale=1.0)
                else:  # yd
                    at = adjp.tile([128, R], U8, tag="adj")
                    nc.sync.dma_start(out=at,
                                      in_=adju8[c * 128:(c + 1) * 128, :])
                    cmin = tq.tile([128, R], F32, tag="t")
                    nc.vector.tensor_scalar(cmin, bc_src, sd_ap(c), 0.0,
                                            OP.add, OP.min)
                    t0 = tq.tile([128, R], F32, tag="t")
                    nc.vector.scalar_tensor_tensor(
                        t0, cmin, -0.8, bc_src, op0=OP.mult, op1=OP.add)
                    t1 = tq.tile([128, R], F32, tag="t")
                    nc.vector.scalar_tensor_tensor(
                        t1, at, -BIG, t0, op0=OP.mult, op1=OP.add)
                    qt = qq.tile([128, R], F32R, tag="q")
                    nc.scalar.activation(qt, t1, AF.Exp, bias=sd_ap(c),
                                         scale=1.0)
                for h2 in range(2):
                    nc.tensor.matmul(
                        u_ps[h2], h_aug[c], qt[:, h2 * 512:(h2 + 1) * 512],
                        start=first, stop=last)

            local_htile(0)
            local_htile(1)

            for c in range(8):
                main_chunk(c)

            for t in range(2, NLOC // 4):
                local_htile(t)
                for c in range(4 * t, 4 * t + 4):
                    main_chunk(c)

            # gathered chunks: g1 first, then g2, loads interleaved
            gchunks = [c for c in range(NLOC, NCHUNK) if _gside(c) == 0] + \
                      [c for c in range(NLOC, NCHUNK) if _gside(c) == 1]
            sd_seen = [False, False]
            for c in gchunks:
                g = _gside(c)
                if not sd_seen[g]:
                    sd_seen[g] = True
                    sd_src = gfull[g][:, F_OUT:F_OUT + 1].rearrange(
                        "(r p) o -> p (r o)", p=128)
                    nc.scalar.dma_start(out=sd_g[g], in_=sd_src)
                row = _grow(c)
                hst = hts.tile([128, FA], F32, tag="hstg", bufs=4,
                               name=f"hstg{c}")
                nc.scalar.dma_start(out=hst, in_=gfull[g][row:row + 128, :])
                nc.gpsimd.tensor_copy(h_aug[c], hst)
                main_chunk(c)

            # ---- epilogue: transpose u.T, normalize, ELU ------------------
            uT_sb = const.tile([FA, R], F32, tag="uT_sb")
            for h2 in range(2):
                nc.vector.tensor_copy(uT_sb[:, h2 * 512:(h2 + 1) * 512],
                                      u_ps[h2])
            for k2 in range(R // 128):
                tr2 = psT.tile([128, FA], F32, tag="tr")
                nc.tensor.transpose(
                    tr2[:, 0:FA],
                    uT_sb[:, k2 * 128:(k2 + 1) * 128],
                    ident[0:FA, 0:FA])
                rc = ep.tile([128, 1], F32, tag="rc")
                nc.vector.reciprocal(rc, tr2[:, F_OUT + 1:FA])
                xs = ep.tile([128, F_OUT], F32, tag="xs")
                nc.vector.tensor_scalar(xs, tr2[:, 0:F_OUT], rc[:, 0:1], None,
                                        OP.mult)
                cm = ep.tile([128, F_OUT], F32, tag="cm")
                nc.vector.tensor_scalar(cm, xs, 0.0, None, OP.min)
                ex = ep.tile([128, F_OUT], F32, tag="ex")
                nc.scalar.activation(ex, cm, AF.Exp, bias=0.0, scale=1.0)
                em = ep.tile([128, F_OUT], F32, tag="em")
                nc.vector.tensor_scalar(em, ex, -1.0, None, OP.add)
                ot = ep.tile([128, F_OUT], F32, tag="ot")
                nc.vector.tensor_tensor(ot, xs, em, OP.max)
                nc.sync.dma_start(out=outd[k2 * 128:(k2 + 1) * 128, :], in_=ot)

    nc.compile()
    return nc


def kernel(input, adj, w, a):
    global LAST_EXEC_NS
    from concourse.bass_utils import run_bass_kernel_spmd

    if "nc" not in _CACHE:
        _CACHE["nc"] = _build()
    nc = _CACHE["nc"]

    input = np.asarray(input, dtype=np.float32)
    adj = np.asarray(adj)
    w = np.asarray(w, dtype=np.float32)
    a = np.asarray(a, dtype=np.float32).reshape(2 * F_OUT)

    inputT = np.ascontiguousarray(input.T)                      # [512, 8192]
    inpre = np.ascontiguousarray(inputT[:, :NLOC * 128])
    wT = np.ascontiguousarray(w.T)                              # [64, 512]
    a2 = np.ascontiguousarray(
        np.stack([a[:F_OUT], a[F_OUT:]], axis=1))               # [64, 2]

    in_maps = []
    for k in range(N_CORES):
        cols = slice(k * R, (k + 1) * R)
        adjc = np.ascontiguousarray(adj[:, cols])
        cmpl = (adjc == 0).astype(np.uint8)                     # complement mask
        mf8 = (cmpl * 0xFB).view(ml_dtypes.float8_e5m2)         # {0, -57344}
        in_maps.append({
            "inpre": inpre,
            "myinT": np.ascontiguousarray(inputT[:, cols]),
            "adju8": cmpl,
            "maskf8": mf8,
            "w": w,
            "wT": wT,
            "a2": a2,
        })

    trace = bool(os.environ.get("GAT_TRACE"))
    res = run_bass_kernel_spmd(nc, in_maps, list(range(N_CORES)), trace=trace)
    LAST_EXEC_NS = res.exec_time_ns
    return np.concatenate([res.results[k]["out"] for k in range(N_CORES)],
                          axis=0)
